# revision 58
# baseline (speedup 1.0000x reference)
"""Trainium2 Bass kernel for multi-head causal attention with RoPE.

Model (per reference):
  B=2, S=2048, D=4096, H=32 heads, HD=128.
  out = softmax(rope(x@wq) @ rope(x@wk)^T / sqrt(HD) + mask) @ (x@wv) @ wo

Sharding: tensor-parallel over heads. Core c in 0..7 owns heads 4c..4c+3:
wq/wk/wv column-sharded, wo row-sharded; each core produces a full-shape
partial output and the host sums the 8 partials (the all-reduce).

Causal fast path — 5 overlapped windows keeping the PE (tensor engine)
stall-free (PE busy is ~1.03 ms of the runtime budget; everything else is
hidden under it):
  W1  QK projections (512-token chunks, 8 PSUM banks), RoPE epilogue with
      PSUM drains split across ACT+DVE, batched spills.
  W2  V projection batch 0 (256-token chunks, 2 PSUM banks).
  W3  V projection batch 1 interleaved with attention(b0).
  W4  attention(b1) interleaved with O-projection(b0).
  W5  O-projection(b1).
Attention per (h,qc) unit: scores with triangle-narrowed diagonal tiles,
exp on ACT, softmax denominator accumulated on DVE into a running tile and
reduced by a single all-ones [128,128] matmul per unit (which also
broadcasts the sums to every partition, so no partition-broadcast op is
needed), PV accumulated in PSUM, normalize via DVE reciprocal+mul.
O-projection drains alternate ACT/DVE (gpsimd cannot touch PSUM) with
fp16 stores. DMAs are batched (multi-dk x loads, whole-row attention
loads, batched rope spills) to respect the ~0.6us/DMA HWDGE
descriptor-generation cost, and large loads avoid the gpsimd SWDGE queue
whose ring holds only ~1k descriptors.
"""

import sys

if "/opt/trn_rl_repo" not in sys.path:
    sys.path.insert(0, "/opt/trn_rl_repo")

import math

import numpy as np

B, S, D, H = 2, 2048, 4096, 32
HD = D // H          # 128
HLOC = 4             # heads per core
NC = 8               # cores
TOK = B * S          # 4096
CH = TOK // 512      # 8 token chunks of 512 (QK phase)
VCH = 8              # 256-token chunks per batch (V phase)
DKT = D // 128       # 32 contraction tiles
QC = S // 512        # 4 q-chunks per sequence
KT = S // 128        # 16 k-tiles per sequence
ISQRT = 1.0 / math.sqrt(HD)

_CACHE = {}


# --------------------------------------------------------------------------
# causal fast path
# --------------------------------------------------------------------------

def _build_causal(nrep: int = 1):
    import concourse.bacc as bacc
    import concourse.tile as tile
    from concourse import mybir

    F32 = mybir.dt.float32
    F32R = mybir.dt.float32r
    F16 = mybir.dt.float16
    EXP = mybir.ActivationFunctionType.Exp

    nc = bacc.Bacc("TRN2", target_bir_lowering=False, debug=False, num_devices=NC)

    xt_d = nc.dram_tensor("xt", [DKT, 128, TOK], F32R, kind="ExternalInput")
    wq_d = nc.dram_tensor("wq", [128, DKT, 512], F32R, kind="ExternalInput")
    wk_d = nc.dram_tensor("wk", [128, DKT, 512], F32R, kind="ExternalInput")
    wv_d = nc.dram_tensor("wv", [128, DKT, 512], F32R, kind="ExternalInput")
    wo_d = nc.dram_tensor("wo", [128, HLOC, D], F32R, kind="ExternalInput")
    cs_d = nc.dram_tensor("cs", [128, S], F32, kind="ExternalInput")
    ss_d = nc.dram_tensor("ss", [128, S], F32, kind="ExternalInput")
    mt_d = nc.dram_tensor("mtri", [128, 128], F32, kind="ExternalInput")
    # fp16 partial output: halves the 64MB store, host sums in fp32
    out_d = nc.dram_tensor("out", [TOK, D], F16, kind="ExternalOutput")

    # DRAM scratch for projected Q/K/V
    qdr = {b: nc.dram_tensor(f"qdr{b}", [HLOC, 128, S], F32R) for b in range(B)}
    kdr = {b: nc.dram_tensor(f"kdr{b}", [HLOC, 128, S], F32R) for b in range(B)}
    vdr = {b: nc.dram_tensor(f"vdr{b}", [S, 512], F32R) for b in range(B)}

    with tile.TileContext(nc) as tc:
        with tc.tile_pool(name="consts", bufs=1) as consts:
            # all-ones [128,128] lhsT: the sums matmul broadcasts the column
            # sums to every partition (same 512-column cost), so no separate
            # partition-broadcast is needed for the normalization
            ones_sb = consts.tile([128, 128], F32R)
            nc.vector.memset(ones_sb.bitcast(F32), 1.0)
            mtri = consts.tile([128, 128], F32, name="mtri")
            nc.scalar.dma_start(out=mtri, in_=mt_d.ap())
            for _ in range(nrep):
                _qk_phase(nc, tc, xt_d, wq_d, wk_d, cs_d, ss_d, qdr, kdr,
                          F32, F32R)
                _vattn_phases(nc, tc, ones_sb, mtri, xt_d, wv_d, wo_d,
                              cs_d, ss_d, qdr, kdr, vdr, out_d,
                              F32, F32R, F16, EXP)

    nc.compile()
    return nc


def _qk_phase(nc, tc, xt_d, wq_d, wk_d, cs_d, ss_d, qdr, kdr, F32, F32R):
    """Q,K projections emitted in transposed [HD, tok] layout with RoPE."""
    with (
        tc.tile_pool(name="w1", bufs=1) as w1,
        tc.tile_pool(name="xt1", bufs=3) as xt1,
        tc.tile_pool(name="rope", bufs=1) as rope,
        tc.tile_pool(name="ps1", bufs=1, space="PSUM") as ps1,
    ):
        wq_sb = w1.tile([128, DKT, 512], F32R, tag="wq")
        wk_sb = w1.tile([128, DKT, 512], F32R, tag="wk")

        for ch in range(CH):
            b, s0 = ch // QC, (ch % QC) * 512
            cs_sb = rope.tile([128, 512], F32, name="cs_c", tag="cs_c", bufs=2)
            ss_sb = rope.tile([128, 512], F32, name="ss_c", tag="ss_c", bufs=2)
            nc.scalar.dma_start(out=cs_sb, in_=cs_d.ap()[:, s0:s0 + 512])
            nc.scalar.dma_start(out=ss_sb, in_=ss_d.ap()[:, s0:s0 + 512])
            qps = [ps1.tile([128, 512], F32, name=f"qps{h}", tag=f"q{h}")
                   for h in range(HLOC)]
            kps = [ps1.tile([128, 512], F32, name=f"kps{h}", tag=f"k{h}")
                   for h in range(HLOC)]
            for g in range(DKT // 2):
                if ch == 0:
                    # weight loads batched 4-dk, just-in-time
                    if g % 2 == 0:
                        dk4 = slice(2 * g, 2 * g + 4)
                        nc.scalar.dma_start(
                            out=wq_sb[:, dk4, :], in_=wq_d.ap()[:, dk4, :]
                        )
                        nc.sync.dma_start(
                            out=wk_sb[:, dk4, :], in_=wk_d.ap()[:, dk4, :]
                        )
                xt = xt1.tile([128, 2, 512], F32R, name="xt", tag="xt")
                nc.sync.dma_start(
                    out=xt,
                    in_=xt_d.ap()[2 * g:2 * g + 2, :,
                                  ch * 512:(ch + 1) * 512].rearrange(
                        "g p t -> p g t"
                    ),
                )
                for gg in range(2):
                    dk = 2 * g + gg
                    for h in range(HLOC):
                        nc.tensor.matmul(
                            qps[h], wq_sb[:, dk, h * 128:(h + 1) * 128],
                            xt[:, gg, :],
                            start=(dk == 0), stop=(dk == DKT - 1),
                        )
                    for h in range(HLOC):
                        nc.tensor.matmul(
                            kps[h], wk_sb[:, dk, h * 128:(h + 1) * 128],
                            xt[:, gg, :],
                            start=(dk == 0), stop=(dk == DKT - 1),
                        )
            # epilogue pass 1: drain all 8 PSUM banks first (frees banks for
            # the next chunk) — copies split across ACT and DVE
            pcs = []
            for i, ps in enumerate(qps + kps):
                pc = rope.tile([128, 512], F32, name="pc", tag="pc", bufs=4)
                if i % 2 == 0:
                    nc.scalar.copy(pc, ps)
                else:
                    nc.vector.tensor_copy(pc, ps)
                pcs.append(pc)
            # pass 2: rope products; s-terms batched per q/k group for the
            # swap DMAs and spills
            tq = rope.tile([128, HLOC, 512], F32, name="tq", tag="tq")
            tk = rope.tile([128, HLOC, 512], F32, name="tk", tag="tk")
            sq = rope.tile([128, HLOC, 512], F32, name="sq", tag="sq")
            sk = rope.tile([128, HLOC, 512], F32, name="sk", tag="sk")
            swq = rope.tile([128, HLOC, 512], F32, name="swq", tag="swq")
            swk = rope.tile([128, HLOC, 512], F32, name="swk", tag="swk")
            for h in range(HLOC):
                nc.vector.tensor_mul(sq[:, h, :], pcs[h], ss_sb)
                nc.vector.tensor_mul(sk[:, h, :], pcs[HLOC + h], ss_sb)
            nc.scalar.dma_start(out=swq[0:64], in_=sq[64:128])
            nc.scalar.dma_start(out=swq[64:128], in_=sq[0:64])
            nc.scalar.dma_start(out=swk[0:64], in_=sk[64:128])
            nc.scalar.dma_start(out=swk[64:128], in_=sk[0:64])
            for h in range(HLOC):
                nc.vector.tensor_mul(tq[:, h, :], pcs[h], cs_sb)
                nc.vector.tensor_mul(tk[:, h, :], pcs[HLOC + h], cs_sb)
            # pass 3: combine + batched spill (one DMA per q/k per chunk)
            for h in range(HLOC):
                nc.vector.tensor_add(tq[:, h, :], tq[:, h, :], swq[:, h, :])
            nc.scalar.dma_start(
                out=qdr[b].ap()[:, :, s0:s0 + 512].rearrange("h p t -> p h t"),
                in_=tq.bitcast(F32R),
            )
            for h in range(HLOC):
                nc.vector.tensor_add(tk[:, h, :], tk[:, h, :], swk[:, h, :])
            nc.scalar.dma_start(
                out=kdr[b].ap()[:, :, s0:s0 + 512].rearrange("h p t -> p h t"),
                in_=tk.bitcast(F32R),
            )


def _vattn_phases(nc, tc, ones_sb, mtri, xt_d, wv_d, wo_d, cs_d, ss_d,
                  qdr, kdr, vdr, out_d, F32, F32R, F16, EXP):
    hbs = [(b, h) for b in range(B) for h in range(HLOC)]

    # pools, LIFO-ordered: attention pools first (live to the end), V pools
    # on top (released after W3); wv/wo on the right side (sequential).
    qkv = tc.alloc_tile_pool(name="qkv", bufs=2)
    hold = tc.alloc_tile_pool(name="hold", bufs=1)
    sm = tc.alloc_tile_pool(name="sm", bufs=2)
    w2 = tc.alloc_tile_pool(name="w2", bufs=1, side="right")     # wv
    xtv = tc.alloc_tile_pool(name="xtv", bufs=4)
    vst = tc.alloc_tile_pool(name="vst", bufs=2)
    # W2 V(b0) uses 512-token chunks (4 banks, alone); W3 V(b1) shares PSUM
    # with attention so it drops to 256-token chunks (2 banks).
    psV0 = tc.alloc_tile_pool(name="psV0", bufs=1, space="PSUM")

    wv_sb = w2.tile([128, DKT, 512], F32R, tag="wv")

    def v_weights(c, g):
        # wv loads ride the sync HWDGE queue (free at the QK->V transition;
        # the gpsimd SWDGE ring only holds ~1k descriptors and would stall).
        # Small first batch so chunk 0's matmuls start early, then 4-dk
        # batches one group ahead of use.
        if c != 0:
            return
        if g == 0:
            for dks in (slice(0, 2), slice(2, 4)):
                nc.sync.dma_start(out=wv_sb[:, dks, :], in_=wv_d.ap()[:, dks, :])
        elif g % 2 == 1 and 2 * g + 2 < DKT:
            dks = slice(2 * g + 2, min(2 * g + 6, DKT))
            nc.sync.dma_start(out=wv_sb[:, dks, :], in_=wv_d.ap()[:, dks, :])

    def v_drain(vps, b, s0, t, eng):
        vc = vst.tile([128, 512], F32R, name="vc", tag="vc")
        if eng is nc.scalar:
            nc.scalar.copy(vc.bitcast(F32), vps)
        else:
            eng.tensor_copy(vc.bitcast(F32), vps)
        nc.gpsimd.dma_start(
            out=vdr[b].ap()[s0 + t * 128:s0 + (t + 1) * 128, :], in_=vc
        )

    def v_chunk0(c):
        # 512-token chunk of batch 0
        s0 = c * 512
        vps = [psV0.tile([128, 512], F32, name=f"v0ps{t}", tag=f"v0{t}")
               for t in range(4)]
        for g in range(DKT // 2):
            v_weights(c, g)
            xg = xtv.tile([128, 2, 512], F32R, name="xg0", tag="xg")
            nc.sync.dma_start(
                out=xg,
                in_=xt_d.ap()[2 * g:2 * g + 2, :, s0:s0 + 512].rearrange(
                    "g p t -> p g t"
                ),
            )
            for gg in range(2):
                dk = 2 * g + gg
                for t in range(4):
                    nc.tensor.matmul(
                        vps[t], xg[:, gg, t * 128:(t + 1) * 128],
                        wv_sb[:, dk, :],
                        start=(dk == 0), stop=(dk == DKT - 1),
                    )
        for t in range(4):
            v_drain(vps[t], 0, s0, t, nc.scalar if t % 2 == 0 else nc.vector)

    def v_chunk1_half(i, psV, vps, hi):
        # half of a 256-token chunk of batch 1 — emitted in two parts around
        # attention units so the PE has V matmuls to run while exps drain
        s0 = i * 256
        if hi == 0:
            vps[:] = [psV.tile([128, 512], F32, name=f"vps{t}", tag=f"v{t}")
                      for t in range(2)]
        for g in range(hi * 4, hi * 4 + 4):
            xg = xtv.tile([128, 4, 256], F32R, name="xg", tag="xg")
            nc.sync.dma_start(
                out=xg,
                in_=xt_d.ap()[4 * g:4 * g + 4, :,
                              S + s0:S + s0 + 256].rearrange(
                    "g p t -> p g t"
                ),
            )
            for gg in range(4):
                dk = 4 * g + gg
                for t in range(2):
                    nc.tensor.matmul(
                        vps[t], xg[:, gg, t * 128:(t + 1) * 128],
                        wv_sb[:, dk, :],
                        start=(dk == 0), stop=(dk == DKT - 1),
                    )
        if hi == 1:
            for t in range(2):
                v_drain(vps[t], 1, s0, t, nc.vector)

    # NOTE: loads must be emitted after the spill writes they read — the tile
    # framework only tracks dependencies on already-emitted instructions.
    # Halved transfers on the scalar/vector HWDGE queues: keeps the sync
    # queue free for JIT xg loads and bounds head-of-line transfer time.
    def load_qk(i):
        b, h = hbs[i]
        qT = qkv.tile([128, S], F32R, name=f"qT{i}", tag="qT")
        kT = qkv.tile([128, S], F32R, name=f"kT{i}", tag="kT")
        for half in range(2):
            sl = slice(half * (S // 2), (half + 1) * (S // 2))
            nc.scalar.dma_start(out=qT[:, sl], in_=qdr[b].ap()[h][:, sl])
            (nc.sync if half == 0 else nc.scalar).dma_start(
                out=kT[:, sl], in_=kdr[b].ap()[h][:, sl]
            )
        return qT, kT

    def load_v(i):
        b, h = hbs[i]
        vT = qkv.tile([128, KT, 128], F32R, name=f"vT{i}", tag="vT")
        vsrc = vdr[b].ap()[:, h * 128:(h + 1) * 128].rearrange(
            "(n p) d -> p n d", p=128
        )
        for half in range(2):
            sl = slice(half * (KT // 2), (half + 1) * (KT // 2))
            (nc.scalar if half == 0 else nc.sync).dma_start(
                out=vT[:, sl, :], in_=vsrc[:, sl, :]
            )
        return vT

    def load_hb(i):
        return (*load_qk(i), load_v(i))

    tiles = {}
    hoTs = {}
    attn_ps = [None, None, None]   # [hops-pool, st-pool, sums-pool]

    def attn_unit(i, qc):
        ps3, ps4, ps5 = attn_ps
        b, h = hbs[i]
        qT, kT, vT = tiles[i]
        hoT = hoTs[b]
        qs = qc * 512
        nkt = (qc + 1) * 4
        hops = ps3.tile([128, 512], F32, name="hops", tag="hops")
        acc = sm.tile([128, 512], F32R, name="acc", tag="acc")
        for kt in range(nkt):
            j = kt - (nkt - 4)
            w = 128 * j if j > 0 else 0
            st = ps4.tile([128, 512], F32, name="st", tag="st")
            nc.tensor.matmul(
                st[:, w:], kT[:, kt * 128:(kt + 1) * 128],
                qT[:, qs + w:qs + 512],
                start=True, stop=True,
            )
            if j >= 0:
                nc.vector.tensor_add(st[:, w:w + 128], st[:, w:w + 128], mtri)
            ex = sm.tile([128, 512], F32R, name="ex", tag="ex", bufs=4)
            nc.scalar.activation(ex[:, w:], st[:, w:], EXP, scale=ISQRT)
            nc.tensor.matmul(
                hops[:, w:], vT[:, kt, :], ex[:, w:],
                start=(kt == 0), stop=(kt == nkt - 1),
            )
            if kt == 0:
                nc.vector.tensor_copy(acc, ex)
            else:
                nc.vector.tensor_add(acc[:, w:], acc[:, w:], ex[:, w:])
        sums = ps5.tile([128, 512], F32, name="sums", tag="sums")
        nc.tensor.matmul(sums, ones_sb, acc, start=True, stop=True)
        rb = sm.tile([128, 512], F32, name="rb", tag="rb", bufs=1)
        nc.vector.reciprocal(rb, sums)
        nc.vector.tensor_mul(hoT[:, h, qs:qs + 512], hops, rb)

    # ---- W2: V projection batch 0, q/k attention loads prefetched ----
    for c in range(4):
        v_chunk0(c)
    psV0.release()
    # attention loads for the first heads go here: late enough that their
    # transfers don't contend with the QK->V transition's spill+weight DMAs,
    # early enough to cover W3's first units
    tiles[0] = load_hb(0)
    tiles[1] = load_hb(1)
    tiles[2] = load_hb(2)

    # ---- W3: V projection batch 1 interleaved with attention(b0) ----
    ps3 = tc.alloc_tile_pool(name="ps3", bufs=2, space="PSUM")   # hops
    ps4 = tc.alloc_tile_pool(name="ps4", bufs=3, space="PSUM")   # st
    ps5 = tc.alloc_tile_pool(name="ps5", bufs=1, space="PSUM")   # sums
    psV = tc.alloc_tile_pool(name="psV", bufs=1, space="PSUM")
    attn_ps[:] = [ps3, ps4, ps5]
    hoTs[0] = hold.tile([128, HLOC, S], F32R, name="hoT0", tag="hoT", bufs=1)
    for i in range(VCH):
        vps = []
        for u in (2 * i, 2 * i + 1):
            v_chunk1_half(i, psV, vps, u % 2)
            h, qc = divmod(u, QC)
            if qc == 0 and h + 1 < HLOC and h + 1 not in tiles:
                tiles[h + 1] = load_hb(h + 1)
            if u == 15:
                # (b1,h0): only now are ALL vdr[1] spill writes emitted
                # (the u==15 half above contains the last chunk's drain)
                tiles[HLOC] = load_hb(HLOC)
            attn_unit(h, qc)

    psV.release()
    vst.release()
    xtv.release()
    w2.release()

    # ---- W4: attention(b1) interleaved with O-projection(b0) ----
    w3p = tc.alloc_tile_pool(name="w3p", bufs=1, side="right")   # wo
    ost = tc.alloc_tile_pool(name="ost", bufs=4)
    psO = tc.alloc_tile_pool(name="psO", bufs=2, space="PSUM")

    wo_sb = w3p.tile([128, HLOC, D], F32R, tag="wo")
    for h in range(HLOC):
        nc.scalar.dma_start(out=wo_sb[:, h, :], in_=wo_d.ap()[:, h, :])

    def o_piece(b, qc, t, half, hoT, three_way):
        # one (t, half) piece: 4 output-column chains + two stores
        c0 = qc * 512 + t * 128
        for pair in range(2):
            ot = ost.tile([128, 2, 512], F16, name="ot", tag="ot")
            for oi in range(2):
                oc = half * 4 + pair * 2 + oi
                if three_way and (pair + oi) % 2 == 1:
                    ops = attn_ps[1].tile([128, 512], F32, name="ops2",
                                          tag="st")
                else:
                    ops = psO.tile([128, 512], F32, name="ops", tag="ops")
                for h in range(HLOC):
                    nc.tensor.matmul(
                        ops, hoT[:, h, c0:c0 + 128],
                        wo_sb[:, h, oc * 512:(oc + 1) * 512],
                        start=(h == 0), stop=(h == HLOC - 1),
                    )
                # drains alternate ACT/DVE (gpsimd cannot access PSUM)
                if (pair * 2 + oi) % 2 == 0:
                    nc.scalar.copy(ot[:, oi, :], ops)
                else:
                    nc.vector.tensor_copy(ot[:, oi, :], ops)
            (nc.scalar if (pair + half) % 2 == 0 else nc.sync).dma_start(
                out=out_d.ap()[b * S + c0:b * S + c0 + 128,
                               (half * 4 + pair * 2) * 512:
                               (half * 4 + pair * 2 + 2) * 512],
                in_=ot,
            )

    def o_block(b, qc, hoT, three_way=False):
        for t in range(4):
            for half in range(2):
                o_piece(b, qc, t, half, hoT, three_way)

    hold1 = tc.alloc_tile_pool(name="hold1", bufs=1, side="right")
    hoTs[1] = hold1.tile([128, HLOC, S], F32R, name="hoT1", tag="hoT1")
    # O(b0) is entirely ready at W4 start: spread its 32 pieces so two
    # follow every attention(b1) unit, filling the exp-latency PE slack
    for u in range(HLOC * QC):
        h, qc = divmod(u, QC)
        i = HLOC + h
        if qc == 0 and h + 1 < HLOC and i + 1 not in tiles:
            tiles[i + 1] = load_hb(i + 1)
        attn_unit(i, qc)
        if qc == 3:
            o_block(0, h, hoTs[0])

    # ---- W5: O-projection(b1) ----
    for qc in range(QC):
        o_block(1, qc, hoTs[1], three_way=True)

    psO.release()
    ost.release()
    hold1.release()
    w3p.release()
    for p in reversed(attn_ps):
        p.release()
    sm.release()
    hold.release()
    qkv.release()


# --------------------------------------------------------------------------
# legacy generic path (non-causal masks)
# --------------------------------------------------------------------------

def _build_legacy():
    import concourse.bacc as bacc
    import concourse.tile as tile
    from concourse import mybir

    F32 = mybir.dt.float32
    F32R = mybir.dt.float32r
    EXP = mybir.ActivationFunctionType.Exp

    nc = bacc.Bacc("TRN2", target_bir_lowering=False, debug=False, num_devices=NC)

    xt_d = nc.dram_tensor("xt", [DKT, 128, TOK], F32R, kind="ExternalInput")
    wq_d = nc.dram_tensor("wq", [128, DKT, 512], F32R, kind="ExternalInput")
    wk_d = nc.dram_tensor("wk", [128, DKT, 512], F32R, kind="ExternalInput")
    wv_d = nc.dram_tensor("wv", [128, DKT, 512], F32R, kind="ExternalInput")
    wo_d = nc.dram_tensor("wo", [128, HLOC, D], F32R, kind="ExternalInput")
    cs_d = nc.dram_tensor("cs", [128, S], F32, kind="ExternalInput")
    ss_d = nc.dram_tensor("ss", [128, S], F32, kind="ExternalInput")
    mk_d = nc.dram_tensor("maskf", [KT, 128, S], F32, kind="ExternalInput")
    out_d = nc.dram_tensor("out", [TOK, D], F32, kind="ExternalOutput")

    qdr = {(h, b): nc.dram_tensor(f"qdr{h}_{b}", [128, S], F32R)
           for h in range(HLOC) for b in range(B)}
    kdr = {(h, b): nc.dram_tensor(f"kdr{h}_{b}", [128, S], F32R)
           for h in range(HLOC) for b in range(B)}
    vdr = {b: nc.dram_tensor(f"vdr{b}", [S, 512], F32R) for b in range(B)}

    with tile.TileContext(nc) as tc:
        with tc.tile_pool(name="consts", bufs=1) as consts:
            ones_sb = consts.tile([128, 1], F32R)
            nc.vector.memset(ones_sb.bitcast(F32), 1.0)

            # Phase 1b: V projection
            with (
                tc.tile_pool(name="w2", bufs=1) as w2,
                tc.tile_pool(name="xt2", bufs=4) as xt2,
                tc.tile_pool(name="vcp", bufs=4) as vcp,
                tc.tile_pool(name="ps2", bufs=2, space="PSUM") as ps2,
            ):
                wv_sb = w2.tile([128, DKT, 512], F32R, tag="wv")
                for ch in range(CH):
                    b, s0 = ch // QC, (ch % QC) * 512
                    vps = [ps2.tile([128, 512], F32, name=f"vps{t}", tag=f"v{t}")
                           for t in range(4)]
                    for dk in range(DKT):
                        if ch == 0:
                            we = nc.scalar if dk % 2 == 0 else nc.sync
                            we.dma_start(out=wv_sb[:, dk, :], in_=wv_d.ap()[:, dk, :])
                        xt = xt2.tile([128, 512], F32R, name="xt", tag="xt")
                        nc.sync.dma_start(
                            out=xt, in_=xt_d.ap()[dk, :, ch * 512:(ch + 1) * 512]
                        )
                        for t in range(4):
                            nc.tensor.matmul(
                                vps[t], xt[:, t * 128:(t + 1) * 128], wv_sb[:, dk, :],
                                start=(dk == 0), stop=(dk == DKT - 1),
                            )
                    for t in range(4):
                        vc = vcp.tile([128, 512], F32R, tag="vc")
                        nc.vector.tensor_copy(vc, vps[t])
                        nc.gpsimd.dma_start(
                            out=vdr[b].ap()[s0 + t * 128:s0 + (t + 1) * 128, :],
                            in_=vc,
                        )

            # Phase 1a: Q,K projections + RoPE
            with (
                tc.tile_pool(name="w1", bufs=1) as w1,
                tc.tile_pool(name="xt1", bufs=4) as xt1,
                tc.tile_pool(name="rope", bufs=2) as rope,
                tc.tile_pool(name="ps1", bufs=1, space="PSUM") as ps1,
            ):
                wq_sb = w1.tile([128, DKT, 512], F32R, tag="wq")
                wk_sb = w1.tile([128, DKT, 512], F32R, tag="wk")
                for ch in range(CH):
                    b, s0 = ch // QC, (ch % QC) * 512
                    cs_sb = rope.tile([128, 512], F32, name="cs_c", tag="cs_c")
                    ss_sb = rope.tile([128, 512], F32, name="ss_c", tag="ss_c")
                    nc.scalar.dma_start(out=cs_sb, in_=cs_d.ap()[:, s0:s0 + 512])
                    nc.scalar.dma_start(out=ss_sb, in_=ss_d.ap()[:, s0:s0 + 512])
                    qps = [ps1.tile([128, 512], F32, name=f"qps{h}", tag=f"q{h}")
                           for h in range(HLOC)]
                    kps = [ps1.tile([128, 512], F32, name=f"kps{h}", tag=f"k{h}")
                           for h in range(HLOC)]
                    for dk in range(DKT):
                        if ch == 0:
                            we = nc.scalar if dk % 2 == 0 else nc.sync
                            wf = nc.sync if dk % 2 == 0 else nc.scalar
                            we.dma_start(out=wq_sb[:, dk, :], in_=wq_d.ap()[:, dk, :])
                            wf.dma_start(out=wk_sb[:, dk, :], in_=wk_d.ap()[:, dk, :])
                        xt = xt1.tile([128, 512], F32R, name="xt", tag="xt")
                        nc.sync.dma_start(
                            out=xt, in_=xt_d.ap()[dk, :, ch * 512:(ch + 1) * 512]
                        )
                        for h in range(HLOC):
                            nc.tensor.matmul(
                                qps[h], wq_sb[:, dk, h * 128:(h + 1) * 128], xt,
                                start=(dk == 0), stop=(dk == DKT - 1),
                            )
                        for h in range(HLOC):
                            nc.tensor.matmul(
                                kps[h], wk_sb[:, dk, h * 128:(h + 1) * 128], xt,
                                start=(dk == 0), stop=(dk == DKT - 1),
                            )
                    work = []
                    for h in range(HLOC):
                        for ps, dst in ((qps[h], qdr), (kps[h], kdr)):
                            pc = rope.tile([128, 512], F32, name="pc", tag="pc",
                                           bufs=4)
                            t1 = rope.tile([128, 512], F32, name="t1", tag="t1",
                                           bufs=8)
                            s1 = rope.tile([128, 512], F32, name="s1", tag="s1",
                                           bufs=2)
                            s1w = rope.tile([128, 512], F32, name="s1w", tag="s1w",
                                            bufs=8)
                            nc.vector.tensor_copy(pc, ps)
                            nc.vector.tensor_mul(t1, pc, cs_sb)
                            nc.vector.tensor_mul(s1, pc, ss_sb)
                            nc.scalar.dma_start(out=s1w[0:64, :], in_=s1[64:128, :])
                            nc.scalar.dma_start(out=s1w[64:128, :], in_=s1[0:64, :])
                            work.append((h, dst, t1, s1w))
                    for h, dst, t1, s1w in work:
                        rr = rope.tile([128, 512], F32R, name="rr", tag="rr", bufs=2)
                        nc.vector.tensor_add(rr, t1, s1w)
                        nc.scalar.dma_start(out=dst[(h, b)].ap()[:, s0:s0 + 512],
                                            in_=rr)

            # Phases 2+3
            hbs = [(b, h) for b in range(B) for h in range(HLOC)]
            with (
                tc.tile_pool(name="qkv", bufs=2) as qkv,
                tc.tile_pool(name="hold", bufs=1) as hold,
                tc.tile_pool(name="smp", bufs=2) as smp,
                tc.tile_pool(name="ps3", bufs=1, space="PSUM") as ps3,
                tc.tile_pool(name="ps4", bufs=3, space="PSUM") as ps4,
            ):
                def load_hb(i):
                    b, h = hbs[i]
                    qT = qkv.tile([128, S], F32R, name=f"qT_{i}", tag="qT")
                    kT = qkv.tile([128, S], F32R, name=f"kT_{i}", tag="kT")
                    vT = qkv.tile([128, KT, 128], F32R, name=f"vT_{i}", tag="vT")
                    vsrc = vdr[b].ap()[:, h * 128:(h + 1) * 128].rearrange(
                        "(n p) d -> p n d", p=128
                    )
                    for j in range(QC):
                        sl = slice(j * 512, (j + 1) * 512)
                        nc.sync.dma_start(out=qT[:, sl], in_=qdr[(h, b)].ap()[:, sl])
                        nc.sync.dma_start(out=kT[:, sl], in_=kdr[(h, b)].ap()[:, sl])
                        nc.sync.dma_start(
                            out=vT[:, j * 4:(j + 1) * 4, :],
                            in_=vsrc[:, j * 4:(j + 1) * 4, :],
                        )
                    return qT, kT, vT

                tiles = {0: load_hb(0)}
                wo_sb = hold.tile([128, HLOC, D], F32R, tag="wo")
                for h in range(HLOC):
                    nc.scalar.dma_start(out=wo_sb[:, h, :], in_=wo_d.ap()[:, h, :])

                hoTs = {}
                for i, (b, h) in enumerate(hbs):
                    if h == 0:
                        hoTs[b] = hold.tile([128, HLOC, S], F32R,
                                            name=f"hoT_{b}", tag=f"hoT{b}")
                    hoT = hoTs[b]
                    if i + 1 < len(hbs):
                        tiles[i + 1] = load_hb(i + 1)
                    qT, kT, vT = tiles.pop(i)
                    for qc in range(QC):
                        qs = qc * 512
                        sums = ps3.tile([1, 512], F32, name="sums", tag="sums")
                        hops = ps3.tile([128, 512], F32, name="hops", tag="hops")
                        for kt in range(KT):
                            st = ps4.tile([128, 512], F32, name="st", tag="st")
                            nc.tensor.matmul(
                                st, kT[:, kt * 128:(kt + 1) * 128],
                                qT[:, qs:qs + 512], start=True, stop=True,
                            )
                            mkt = smp.tile([128, 512], F32, name="mkt", tag="mkt")
                            nc.sync.dma_start(out=mkt, in_=mk_d.ap()[kt, :, qs:qs + 512])
                            nc.vector.tensor_add(st, st, mkt)
                            ex = smp.tile([128, 512], F32R, name="ex", tag="ex",
                                          bufs=4)
                            nc.scalar.activation(ex, st, EXP, scale=ISQRT)
                            nc.tensor.matmul(sums, ones_sb, ex, start=(kt == 0),
                                             stop=(kt == KT - 1))
                            nc.tensor.matmul(hops, vT[:, kt, :], ex, start=(kt == 0),
                                             stop=(kt == KT - 1))
                        recip = smp.tile([1, 512], F32, name="recip", tag="recip")
                        nc.vector.reciprocal(recip, sums)
                        bc = smp.tile([128, 512], F32, name="bc", tag="bc")
                        nc.gpsimd.partition_broadcast(bc, recip)
                        nc.vector.tensor_mul(hoT[:, h, qs:qs + 512], hops, bc)

                for b in range(B):
                    with (
                        tc.tile_pool(name=f"oc{b}", bufs=3) as ocp,
                        tc.tile_pool(name=f"ps5{b}", bufs=3, space="PSUM") as ps5,
                    ):
                        for t in range(S // 128):
                            for oc in range(D // 512):
                                ops = ps5.tile([128, 512], F32, name="ops", tag="ops")
                                for h in range(HLOC):
                                    nc.tensor.matmul(
                                        ops, hoTs[b][:, h, t * 128:(t + 1) * 128],
                                        wo_sb[:, h, oc * 512:(oc + 1) * 512],
                                        start=(h == 0), stop=(h == HLOC - 1),
                                    )
                                ot = ocp.tile([128, 512], F32, name="ot", tag="ot")
                                nc.vector.tensor_copy(ot, ops)
                                nc.scalar.dma_start(
                                    out=out_d.ap()[
                                        b * S + t * 128:b * S + (t + 1) * 128,
                                        oc * 512:(oc + 1) * 512,
                                    ],
                                    in_=ot,
                                )

    nc.compile()
    return nc


def _get_nc(causal: bool):
    if causal not in _CACHE:
        _CACHE[causal] = _build_causal() if causal else _build_legacy()
    return _CACHE[causal]


def _host_prep(x, wq, wk, wv, wo, freqs_cos, freqs_sin, mask):
    """Build per-core input maps."""
    x2 = np.ascontiguousarray(x.reshape(TOK, D).T)          # [D, TOK]
    xt = x2.reshape(DKT, 128, TOK)

    cs = np.concatenate([freqs_cos.T, freqs_cos.T], axis=0).astype(np.float32)
    ss = np.concatenate([freqs_sin.T, -freqs_sin.T], axis=0).astype(np.float32)

    m2 = np.asarray(mask, dtype=np.float32).reshape(S, S)
    tril = np.tril(np.ones((S, S), dtype=bool))
    causal = bool(np.all(m2[tril] == 0.0) and np.all(m2[~tril] <= -1e8))
    if causal:
        mk = np.ascontiguousarray(m2[:128, :128].T)         # [k,q] triangle
    else:
        mk = np.ascontiguousarray(m2.T.reshape(KT, 128, S))

    # per-head column permutation: evens then odds (RoPE rotate-half form)
    perm = np.concatenate([np.arange(0, HD, 2), np.arange(1, HD, 2)])

    in_maps = []
    for c in range(NC):
        cols = np.concatenate(
            [(4 * c + h) * HD + perm for h in range(HLOC)]
        )
        wq_c = np.ascontiguousarray(
            wq[:, cols].reshape(DKT, 128, 512).transpose(1, 0, 2)
        )
        wk_c = np.ascontiguousarray(
            wk[:, cols].reshape(DKT, 128, 512).transpose(1, 0, 2)
        )
        vcols = np.arange(4 * c * HD, 4 * (c + 1) * HD)
        wv_c = np.ascontiguousarray(
            wv[:, vcols].reshape(DKT, 128, 512).transpose(1, 0, 2)
        )
        wo_c = np.ascontiguousarray(
            wo[vcols, :].reshape(HLOC, 128, D).transpose(1, 0, 2)
        )
        m = {
            "xt": xt, "wq": wq_c, "wk": wk_c, "wv": wv_c, "wo": wo_c,
            "cs": cs, "ss": ss,
        }
        m["mtri" if causal else "maskf"] = mk
        in_maps.append(m)
    return in_maps, causal


def kernel(x, wq, wk, wv, wo, freqs_cos, freqs_sin, mask, **_unused):
    from concourse.bass_utils import run_bass_kernel_spmd

    x = np.asarray(x, dtype=np.float32)
    wq = np.asarray(wq, dtype=np.float32)
    wk = np.asarray(wk, dtype=np.float32)
    wv = np.asarray(wv, dtype=np.float32)
    wo = np.asarray(wo, dtype=np.float32)
    freqs_cos = np.asarray(freqs_cos, dtype=np.float32)
    freqs_sin = np.asarray(freqs_sin, dtype=np.float32)

    in_maps, causal = _host_prep(x, wq, wk, wv, wo, freqs_cos, freqs_sin, mask)
    nc = _get_nc(causal)
    res = run_bass_kernel_spmd(nc, in_maps, list(range(NC)))
    out = res.results[0]["out"].astype(np.float32)
    for c in range(1, NC):
        out = out + res.results[c]["out"].astype(np.float32)
    return out.reshape(B, S, D).astype(np.float32)


# revision 59
# speedup vs baseline: 1.0016x; 1.0016x over previous
"""Trainium2 Bass kernel for multi-head causal attention with RoPE.

Model (per reference):
  B=2, S=2048, D=4096, H=32 heads, HD=128.
  out = softmax(rope(x@wq) @ rope(x@wk)^T / sqrt(HD) + mask) @ (x@wv) @ wo

Sharding: tensor-parallel over heads. Core c in 0..7 owns heads 4c..4c+3:
wq/wk/wv column-sharded, wo row-sharded; each core produces a full-shape
partial output and the host sums the 8 partials (the all-reduce).

Causal fast path — 5 overlapped windows keeping the PE (tensor engine)
stall-free (PE busy is ~1.03 ms of the runtime budget; everything else is
hidden under it):
  W1  QK projections (512-token chunks, 8 PSUM banks), RoPE epilogue with
      PSUM drains split across ACT+DVE, batched spills.
  W2  V projection batch 0 (256-token chunks, 2 PSUM banks).
  W3  V projection batch 1 interleaved with attention(b0).
  W4  attention(b1) interleaved with O-projection(b0).
  W5  O-projection(b1).
Attention per (h,qc) unit: scores with triangle-narrowed diagonal tiles,
exp on ACT, softmax denominator accumulated on DVE into a running tile and
reduced by a single all-ones [128,128] matmul per unit (which also
broadcasts the sums to every partition, so no partition-broadcast op is
needed), PV accumulated in PSUM, normalize via DVE reciprocal+mul.
O-projection drains alternate ACT/DVE (gpsimd cannot touch PSUM) with
fp16 stores. DMAs are batched (multi-dk x loads, whole-row attention
loads, batched rope spills) to respect the ~0.6us/DMA HWDGE
descriptor-generation cost, and large loads avoid the gpsimd SWDGE queue
whose ring holds only ~1k descriptors.
"""

import sys

if "/opt/trn_rl_repo" not in sys.path:
    sys.path.insert(0, "/opt/trn_rl_repo")

import math

import numpy as np

B, S, D, H = 2, 2048, 4096, 32
HD = D // H          # 128
HLOC = 4             # heads per core
NC = 8               # cores
TOK = B * S          # 4096
CH = TOK // 512      # 8 token chunks of 512 (QK phase)
VCH = 8              # 256-token chunks per batch (V phase)
DKT = D // 128       # 32 contraction tiles
QC = S // 512        # 4 q-chunks per sequence
KT = S // 128        # 16 k-tiles per sequence
ISQRT = 1.0 / math.sqrt(HD)

_CACHE = {}


# --------------------------------------------------------------------------
# causal fast path
# --------------------------------------------------------------------------

def _build_causal(nrep: int = 1):
    import concourse.bacc as bacc
    import concourse.tile as tile
    from concourse import mybir

    F32 = mybir.dt.float32
    F32R = mybir.dt.float32r
    F16 = mybir.dt.float16
    EXP = mybir.ActivationFunctionType.Exp

    nc = bacc.Bacc("TRN2", target_bir_lowering=False, debug=False, num_devices=NC)

    xt_d = nc.dram_tensor("xt", [DKT, 128, TOK], F32R, kind="ExternalInput")
    wq_d = nc.dram_tensor("wq", [128, DKT, 512], F32R, kind="ExternalInput")
    wk_d = nc.dram_tensor("wk", [128, DKT, 512], F32R, kind="ExternalInput")
    wv_d = nc.dram_tensor("wv", [128, DKT, 512], F32R, kind="ExternalInput")
    wo_d = nc.dram_tensor("wo", [128, HLOC, D], F32R, kind="ExternalInput")
    cs_d = nc.dram_tensor("cs", [128, S], F32, kind="ExternalInput")
    ss_d = nc.dram_tensor("ss", [128, S], F32, kind="ExternalInput")
    mt_d = nc.dram_tensor("mtri", [128, 128], F32, kind="ExternalInput")
    # fp16 partial output: halves the 64MB store, host sums in fp32
    out_d = nc.dram_tensor("out", [TOK, D], F16, kind="ExternalOutput")

    # DRAM scratch for projected Q/K/V
    qdr = {b: nc.dram_tensor(f"qdr{b}", [HLOC, 128, S], F32R) for b in range(B)}
    kdr = {b: nc.dram_tensor(f"kdr{b}", [HLOC, 128, S], F32R) for b in range(B)}
    vdr = {b: nc.dram_tensor(f"vdr{b}", [S, 512], F32R) for b in range(B)}

    with tile.TileContext(nc) as tc:
        with tc.tile_pool(name="consts", bufs=1) as consts:
            # all-ones [128,128] lhsT: the sums matmul broadcasts the column
            # sums to every partition (same 512-column cost), so no separate
            # partition-broadcast is needed for the normalization
            ones_sb = consts.tile([128, 128], F32R)
            nc.vector.memset(ones_sb.bitcast(F32), 1.0)
            mtri = consts.tile([128, 128], F32, name="mtri")
            nc.scalar.dma_start(out=mtri, in_=mt_d.ap())
            for _ in range(nrep):
                _qk_phase(nc, tc, xt_d, wq_d, wk_d, cs_d, ss_d, qdr, kdr,
                          F32, F32R)
                _vattn_phases(nc, tc, ones_sb, mtri, xt_d, wv_d, wo_d,
                              cs_d, ss_d, qdr, kdr, vdr, out_d,
                              F32, F32R, F16, EXP)

    nc.compile()
    return nc


def _qk_phase(nc, tc, xt_d, wq_d, wk_d, cs_d, ss_d, qdr, kdr, F32, F32R):
    """Q,K projections emitted in transposed [HD, tok] layout with RoPE."""
    with (
        tc.tile_pool(name="w1", bufs=1) as w1,
        tc.tile_pool(name="xt1", bufs=3) as xt1,
        tc.tile_pool(name="rope", bufs=1) as rope,
        tc.tile_pool(name="ps1", bufs=1, space="PSUM") as ps1,
    ):
        wq_sb = w1.tile([128, DKT, 512], F32R, tag="wq")
        wk_sb = w1.tile([128, DKT, 512], F32R, tag="wk")

        for ch in range(CH):
            b, s0 = ch // QC, (ch % QC) * 512
            cs_sb = rope.tile([128, 512], F32, name="cs_c", tag="cs_c", bufs=2)
            ss_sb = rope.tile([128, 512], F32, name="ss_c", tag="ss_c", bufs=2)
            nc.scalar.dma_start(out=cs_sb, in_=cs_d.ap()[:, s0:s0 + 512])
            nc.scalar.dma_start(out=ss_sb, in_=ss_d.ap()[:, s0:s0 + 512])
            qps = [ps1.tile([128, 512], F32, name=f"qps{h}", tag=f"q{h}")
                   for h in range(HLOC)]
            kps = [ps1.tile([128, 512], F32, name=f"kps{h}", tag=f"k{h}")
                   for h in range(HLOC)]
            for g in range(DKT // 2):
                if ch == 0:
                    # weight loads batched 4-dk, just-in-time
                    if g % 2 == 0:
                        dk4 = slice(2 * g, 2 * g + 4)
                        nc.scalar.dma_start(
                            out=wq_sb[:, dk4, :], in_=wq_d.ap()[:, dk4, :]
                        )
                        nc.sync.dma_start(
                            out=wk_sb[:, dk4, :], in_=wk_d.ap()[:, dk4, :]
                        )
                xt = xt1.tile([128, 2, 512], F32R, name="xt", tag="xt")
                nc.sync.dma_start(
                    out=xt,
                    in_=xt_d.ap()[2 * g:2 * g + 2, :,
                                  ch * 512:(ch + 1) * 512].rearrange(
                        "g p t -> p g t"
                    ),
                )
                for gg in range(2):
                    dk = 2 * g + gg
                    for h in range(HLOC):
                        nc.tensor.matmul(
                            qps[h], wq_sb[:, dk, h * 128:(h + 1) * 128],
                            xt[:, gg, :],
                            start=(dk == 0), stop=(dk == DKT - 1),
                        )
                    for h in range(HLOC):
                        nc.tensor.matmul(
                            kps[h], wk_sb[:, dk, h * 128:(h + 1) * 128],
                            xt[:, gg, :],
                            start=(dk == 0), stop=(dk == DKT - 1),
                        )
            # epilogue pass 1: drain all 8 PSUM banks first (frees banks for
            # the next chunk) — copies split across ACT and DVE
            pcs = []
            for i, ps in enumerate(qps + kps):
                pc = rope.tile([128, 512], F32, name="pc", tag="pc", bufs=4)
                if i % 2 == 0:
                    nc.scalar.copy(pc, ps)
                else:
                    nc.vector.tensor_copy(pc, ps)
                pcs.append(pc)
            # pass 2: rope products; s-terms batched per q/k group for the
            # swap DMAs and spills
            tq = rope.tile([128, HLOC, 512], F32, name="tq", tag="tq")
            tk = rope.tile([128, HLOC, 512], F32, name="tk", tag="tk")
            sq = rope.tile([128, HLOC, 512], F32, name="sq", tag="sq")
            sk = rope.tile([128, HLOC, 512], F32, name="sk", tag="sk")
            swq = rope.tile([128, HLOC, 512], F32, name="swq", tag="swq")
            swk = rope.tile([128, HLOC, 512], F32, name="swk", tag="swk")
            for h in range(HLOC):
                nc.vector.tensor_mul(sq[:, h, :], pcs[h], ss_sb)
                nc.vector.tensor_mul(sk[:, h, :], pcs[HLOC + h], ss_sb)
            nc.scalar.dma_start(out=swq[0:64], in_=sq[64:128])
            nc.scalar.dma_start(out=swq[64:128], in_=sq[0:64])
            nc.scalar.dma_start(out=swk[0:64], in_=sk[64:128])
            nc.scalar.dma_start(out=swk[64:128], in_=sk[0:64])
            for h in range(HLOC):
                nc.vector.tensor_mul(tq[:, h, :], pcs[h], cs_sb)
                nc.vector.tensor_mul(tk[:, h, :], pcs[HLOC + h], cs_sb)
            # pass 3: combine + batched spill (one DMA per q/k per chunk)
            for h in range(HLOC):
                nc.vector.tensor_add(tq[:, h, :], tq[:, h, :], swq[:, h, :])
            nc.scalar.dma_start(
                out=qdr[b].ap()[:, :, s0:s0 + 512].rearrange("h p t -> p h t"),
                in_=tq.bitcast(F32R),
            )
            for h in range(HLOC):
                nc.vector.tensor_add(tk[:, h, :], tk[:, h, :], swk[:, h, :])
            nc.scalar.dma_start(
                out=kdr[b].ap()[:, :, s0:s0 + 512].rearrange("h p t -> p h t"),
                in_=tk.bitcast(F32R),
            )


def _vattn_phases(nc, tc, ones_sb, mtri, xt_d, wv_d, wo_d, cs_d, ss_d,
                  qdr, kdr, vdr, out_d, F32, F32R, F16, EXP):
    hbs = [(b, h) for b in range(B) for h in range(HLOC)]

    # pools, LIFO-ordered: attention pools first (live to the end), V pools
    # on top (released after W3); wv/wo on the right side (sequential).
    qkv = tc.alloc_tile_pool(name="qkv", bufs=2)
    hold = tc.alloc_tile_pool(name="hold", bufs=1)
    sm = tc.alloc_tile_pool(name="sm", bufs=2)
    w2 = tc.alloc_tile_pool(name="w2", bufs=1, side="right")     # wv
    xtv = tc.alloc_tile_pool(name="xtv", bufs=4)
    vst = tc.alloc_tile_pool(name="vst", bufs=2)
    # W2 V(b0) uses 512-token chunks (4 banks, alone); W3 V(b1) shares PSUM
    # with attention so it drops to 256-token chunks (2 banks).
    psV0 = tc.alloc_tile_pool(name="psV0", bufs=1, space="PSUM")

    wv_sb = w2.tile([128, DKT - 6, 512], F32R, tag="wv")
    # first 6 dk-tiles of wv staged in the xtv pool: its space sits in the
    # freed w1/xt1 range (NOT gated by the rope-pool release), so these
    # loads transfer during the last QK chunk's epilogue and V(b0) starts
    # ~15us earlier; xtv dies at W3-end so no W4 budget is consumed.
    wvs = xtv.tile([128, 6, 512], F32R, tag="wvs", bufs=1)
    for dks in (slice(0, 3), slice(3, 6)):
        nc.sync.dma_start(out=wvs[:, dks, :], in_=wv_d.ap()[:, dks, :])

    def wv_at(dk):
        return wvs[:, dk, :] if dk < 6 else wv_sb[:, dk - 6, :]

    def v_weights(c, g):
        # remaining wv tiles (dk 6-31) on the sync HWDGE queue, batched one
        # group ahead of use (these ARE gated by the rope release)
        if c != 0 or g % 2 != 1:
            return
        if g == 1:
            nc.sync.dma_start(out=wv_sb[:, 0:2, :], in_=wv_d.ap()[:, 6:8, :])
        lo = 2 * g + 2
        if 8 <= lo < DKT:
            nc.sync.dma_start(
                out=wv_sb[:, lo - 6:lo - 2, :], in_=wv_d.ap()[:, lo:lo + 4, :]
            )

    def v_drain(vps, b, s0, t, eng):
        vc = vst.tile([128, 512], F32R, name="vc", tag="vc")
        if eng is nc.scalar:
            nc.scalar.copy(vc.bitcast(F32), vps)
        else:
            eng.tensor_copy(vc.bitcast(F32), vps)
        nc.gpsimd.dma_start(
            out=vdr[b].ap()[s0 + t * 128:s0 + (t + 1) * 128, :], in_=vc
        )

    def v_chunk0(c):
        # 512-token chunk of batch 0
        s0 = c * 512
        vps = [psV0.tile([128, 512], F32, name=f"v0ps{t}", tag=f"v0{t}")
               for t in range(4)]
        for g in range(DKT // 2):
            v_weights(c, g)
            xg = xtv.tile([128, 2, 512], F32R, name="xg0", tag="xg")
            nc.sync.dma_start(
                out=xg,
                in_=xt_d.ap()[2 * g:2 * g + 2, :, s0:s0 + 512].rearrange(
                    "g p t -> p g t"
                ),
            )
            for gg in range(2):
                dk = 2 * g + gg
                for t in range(4):
                    nc.tensor.matmul(
                        vps[t], xg[:, gg, t * 128:(t + 1) * 128],
                        wv_at(dk),
                        start=(dk == 0), stop=(dk == DKT - 1),
                    )
        for t in range(4):
            v_drain(vps[t], 0, s0, t, nc.scalar if t % 2 == 0 else nc.vector)

    def v_chunk1_half(i, psV, vps, hi):
        # half of a 256-token chunk of batch 1 — emitted in two parts around
        # attention units so the PE has V matmuls to run while exps drain
        s0 = i * 256
        if hi == 0:
            vps[:] = [psV.tile([128, 512], F32, name=f"vps{t}", tag=f"v{t}")
                      for t in range(2)]
        for g in range(hi * 4, hi * 4 + 4):
            xg = xtv.tile([128, 4, 256], F32R, name="xg", tag="xg")
            nc.sync.dma_start(
                out=xg,
                in_=xt_d.ap()[4 * g:4 * g + 4, :,
                              S + s0:S + s0 + 256].rearrange(
                    "g p t -> p g t"
                ),
            )
            for gg in range(4):
                dk = 4 * g + gg
                for t in range(2):
                    nc.tensor.matmul(
                        vps[t], xg[:, gg, t * 128:(t + 1) * 128],
                        wv_at(dk),
                        start=(dk == 0), stop=(dk == DKT - 1),
                    )
        if hi == 1:
            for t in range(2):
                v_drain(vps[t], 1, s0, t, nc.vector)

    # NOTE: loads must be emitted after the spill writes they read — the tile
    # framework only tracks dependencies on already-emitted instructions.
    # Halved transfers on the scalar/vector HWDGE queues: keeps the sync
    # queue free for JIT xg loads and bounds head-of-line transfer time.
    def load_qk(i):
        b, h = hbs[i]
        qT = qkv.tile([128, S], F32R, name=f"qT{i}", tag="qT")
        kT = qkv.tile([128, S], F32R, name=f"kT{i}", tag="kT")
        for half in range(2):
            sl = slice(half * (S // 2), (half + 1) * (S // 2))
            nc.scalar.dma_start(out=qT[:, sl], in_=qdr[b].ap()[h][:, sl])
            (nc.sync if half == 0 else nc.scalar).dma_start(
                out=kT[:, sl], in_=kdr[b].ap()[h][:, sl]
            )
        return qT, kT

    def load_v(i):
        b, h = hbs[i]
        vT = qkv.tile([128, KT, 128], F32R, name=f"vT{i}", tag="vT")
        vsrc = vdr[b].ap()[:, h * 128:(h + 1) * 128].rearrange(
            "(n p) d -> p n d", p=128
        )
        for half in range(2):
            sl = slice(half * (KT // 2), (half + 1) * (KT // 2))
            (nc.scalar if half == 0 else nc.sync).dma_start(
                out=vT[:, sl, :], in_=vsrc[:, sl, :]
            )
        return vT

    def load_hb(i):
        return (*load_qk(i), load_v(i))

    tiles = {}
    hoTs = {}
    attn_ps = [None, None, None]   # [hops-pool, st-pool, sums-pool]

    def attn_unit(i, qc):
        ps3, ps4, ps5 = attn_ps
        b, h = hbs[i]
        qT, kT, vT = tiles[i]
        hoT = hoTs[b]
        qs = qc * 512
        nkt = (qc + 1) * 4
        hops = ps3.tile([128, 512], F32, name="hops", tag="hops")
        acc = sm.tile([128, 512], F32R, name="acc", tag="acc")
        for kt in range(nkt):
            j = kt - (nkt - 4)
            w = 128 * j if j > 0 else 0
            st = ps4.tile([128, 512], F32, name="st", tag="st")
            nc.tensor.matmul(
                st[:, w:], kT[:, kt * 128:(kt + 1) * 128],
                qT[:, qs + w:qs + 512],
                start=True, stop=True,
            )
            if j >= 0:
                nc.vector.tensor_add(st[:, w:w + 128], st[:, w:w + 128], mtri)
            ex = sm.tile([128, 512], F32R, name="ex", tag="ex", bufs=4)
            nc.scalar.activation(ex[:, w:], st[:, w:], EXP, scale=ISQRT)
            nc.tensor.matmul(
                hops[:, w:], vT[:, kt, :], ex[:, w:],
                start=(kt == 0), stop=(kt == nkt - 1),
            )
            if kt == 0:
                nc.vector.tensor_copy(acc, ex)
            else:
                nc.vector.tensor_add(acc[:, w:], acc[:, w:], ex[:, w:])
        sums = ps5.tile([128, 512], F32, name="sums", tag="sums")
        nc.tensor.matmul(sums, ones_sb, acc, start=True, stop=True)
        rb = sm.tile([128, 512], F32, name="rb", tag="rb", bufs=1)
        nc.vector.reciprocal(rb, sums)
        nc.vector.tensor_mul(hoT[:, h, qs:qs + 512], hops, rb)

    # ---- W2: V projection batch 0, q/k attention loads prefetched ----
    for c in range(4):
        v_chunk0(c)
    psV0.release()
    # attention loads for the first heads go here: late enough that their
    # transfers don't contend with the QK->V transition's spill+weight DMAs,
    # early enough to cover W3's first units
    tiles[0] = load_hb(0)
    tiles[1] = load_hb(1)
    tiles[2] = load_hb(2)

    # ---- W3: V projection batch 1 interleaved with attention(b0) ----
    ps3 = tc.alloc_tile_pool(name="ps3", bufs=2, space="PSUM")   # hops
    ps4 = tc.alloc_tile_pool(name="ps4", bufs=3, space="PSUM")   # st
    ps5 = tc.alloc_tile_pool(name="ps5", bufs=1, space="PSUM")   # sums
    psV = tc.alloc_tile_pool(name="psV", bufs=1, space="PSUM")
    attn_ps[:] = [ps3, ps4, ps5]
    hoTs[0] = hold.tile([128, HLOC, S], F32R, name="hoT0", tag="hoT", bufs=1)
    for i in range(VCH):
        vps = []
        for u in (2 * i, 2 * i + 1):
            v_chunk1_half(i, psV, vps, u % 2)
            h, qc = divmod(u, QC)
            if qc == 0 and h + 1 < HLOC and h + 1 not in tiles:
                tiles[h + 1] = load_hb(h + 1)
            if u == 15:
                # (b1,h0): only now are ALL vdr[1] spill writes emitted
                # (the u==15 half above contains the last chunk's drain)
                tiles[HLOC] = load_hb(HLOC)
            attn_unit(h, qc)

    psV.release()
    vst.release()
    xtv.release()
    w2.release()

    # ---- W4: attention(b1) interleaved with O-projection(b0) ----
    w3p = tc.alloc_tile_pool(name="w3p", bufs=1, side="right")   # wo
    ost = tc.alloc_tile_pool(name="ost", bufs=4)
    psO = tc.alloc_tile_pool(name="psO", bufs=2, space="PSUM")

    wo_sb = w3p.tile([128, HLOC, D], F32R, tag="wo")
    for h in range(HLOC):
        nc.scalar.dma_start(out=wo_sb[:, h, :], in_=wo_d.ap()[:, h, :])

    def o_piece(b, qc, t, half, hoT, three_way):
        # one (t, half) piece: 4 output-column chains + two stores
        c0 = qc * 512 + t * 128
        for pair in range(2):
            ot = ost.tile([128, 2, 512], F16, name="ot", tag="ot")
            for oi in range(2):
                oc = half * 4 + pair * 2 + oi
                if three_way and (pair + oi) % 2 == 1:
                    ops = attn_ps[1].tile([128, 512], F32, name="ops2",
                                          tag="st")
                else:
                    ops = psO.tile([128, 512], F32, name="ops", tag="ops")
                for h in range(HLOC):
                    nc.tensor.matmul(
                        ops, hoT[:, h, c0:c0 + 128],
                        wo_sb[:, h, oc * 512:(oc + 1) * 512],
                        start=(h == 0), stop=(h == HLOC - 1),
                    )
                # drains alternate ACT/DVE (gpsimd cannot access PSUM)
                if (pair * 2 + oi) % 2 == 0:
                    nc.scalar.copy(ot[:, oi, :], ops)
                else:
                    nc.vector.tensor_copy(ot[:, oi, :], ops)
            (nc.scalar if (pair + half) % 2 == 0 else nc.sync).dma_start(
                out=out_d.ap()[b * S + c0:b * S + c0 + 128,
                               (half * 4 + pair * 2) * 512:
                               (half * 4 + pair * 2 + 2) * 512],
                in_=ot,
            )

    def o_block(b, qc, hoT, three_way=False):
        for t in range(4):
            for half in range(2):
                o_piece(b, qc, t, half, hoT, three_way)

    hold1 = tc.alloc_tile_pool(name="hold1", bufs=1, side="right")
    hoTs[1] = hold1.tile([128, HLOC, S], F32R, name="hoT1", tag="hoT1")
    # O(b0) is entirely ready at W4 start: spread its 32 pieces so two
    # follow every attention(b1) unit, filling the exp-latency PE slack
    for u in range(HLOC * QC):
        h, qc = divmod(u, QC)
        i = HLOC + h
        if qc == 0 and h + 1 < HLOC and i + 1 not in tiles:
            tiles[i + 1] = load_hb(i + 1)
        attn_unit(i, qc)
        if qc == 3:
            o_block(0, h, hoTs[0])

    # ---- W5: O-projection(b1) ----
    for qc in range(QC):
        o_block(1, qc, hoTs[1], three_way=True)

    psO.release()
    ost.release()
    hold1.release()
    w3p.release()
    for p in reversed(attn_ps):
        p.release()
    sm.release()
    hold.release()
    qkv.release()


# --------------------------------------------------------------------------
# legacy generic path (non-causal masks)
# --------------------------------------------------------------------------

def _build_legacy():
    import concourse.bacc as bacc
    import concourse.tile as tile
    from concourse import mybir

    F32 = mybir.dt.float32
    F32R = mybir.dt.float32r
    EXP = mybir.ActivationFunctionType.Exp

    nc = bacc.Bacc("TRN2", target_bir_lowering=False, debug=False, num_devices=NC)

    xt_d = nc.dram_tensor("xt", [DKT, 128, TOK], F32R, kind="ExternalInput")
    wq_d = nc.dram_tensor("wq", [128, DKT, 512], F32R, kind="ExternalInput")
    wk_d = nc.dram_tensor("wk", [128, DKT, 512], F32R, kind="ExternalInput")
    wv_d = nc.dram_tensor("wv", [128, DKT, 512], F32R, kind="ExternalInput")
    wo_d = nc.dram_tensor("wo", [128, HLOC, D], F32R, kind="ExternalInput")
    cs_d = nc.dram_tensor("cs", [128, S], F32, kind="ExternalInput")
    ss_d = nc.dram_tensor("ss", [128, S], F32, kind="ExternalInput")
    mk_d = nc.dram_tensor("maskf", [KT, 128, S], F32, kind="ExternalInput")
    out_d = nc.dram_tensor("out", [TOK, D], F32, kind="ExternalOutput")

    qdr = {(h, b): nc.dram_tensor(f"qdr{h}_{b}", [128, S], F32R)
           for h in range(HLOC) for b in range(B)}
    kdr = {(h, b): nc.dram_tensor(f"kdr{h}_{b}", [128, S], F32R)
           for h in range(HLOC) for b in range(B)}
    vdr = {b: nc.dram_tensor(f"vdr{b}", [S, 512], F32R) for b in range(B)}

    with tile.TileContext(nc) as tc:
        with tc.tile_pool(name="consts", bufs=1) as consts:
            ones_sb = consts.tile([128, 1], F32R)
            nc.vector.memset(ones_sb.bitcast(F32), 1.0)

            # Phase 1b: V projection
            with (
                tc.tile_pool(name="w2", bufs=1) as w2,
                tc.tile_pool(name="xt2", bufs=4) as xt2,
                tc.tile_pool(name="vcp", bufs=4) as vcp,
                tc.tile_pool(name="ps2", bufs=2, space="PSUM") as ps2,
            ):
                wv_sb = w2.tile([128, DKT, 512], F32R, tag="wv")
                for ch in range(CH):
                    b, s0 = ch // QC, (ch % QC) * 512
                    vps = [ps2.tile([128, 512], F32, name=f"vps{t}", tag=f"v{t}")
                           for t in range(4)]
                    for dk in range(DKT):
                        if ch == 0:
                            we = nc.scalar if dk % 2 == 0 else nc.sync
                            we.dma_start(out=wv_sb[:, dk, :], in_=wv_d.ap()[:, dk, :])
                        xt = xt2.tile([128, 512], F32R, name="xt", tag="xt")
                        nc.sync.dma_start(
                            out=xt, in_=xt_d.ap()[dk, :, ch * 512:(ch + 1) * 512]
                        )
                        for t in range(4):
                            nc.tensor.matmul(
                                vps[t], xt[:, t * 128:(t + 1) * 128], wv_sb[:, dk, :],
                                start=(dk == 0), stop=(dk == DKT - 1),
                            )
                    for t in range(4):
                        vc = vcp.tile([128, 512], F32R, tag="vc")
                        nc.vector.tensor_copy(vc, vps[t])
                        nc.gpsimd.dma_start(
                            out=vdr[b].ap()[s0 + t * 128:s0 + (t + 1) * 128, :],
                            in_=vc,
                        )

            # Phase 1a: Q,K projections + RoPE
            with (
                tc.tile_pool(name="w1", bufs=1) as w1,
                tc.tile_pool(name="xt1", bufs=4) as xt1,
                tc.tile_pool(name="rope", bufs=2) as rope,
                tc.tile_pool(name="ps1", bufs=1, space="PSUM") as ps1,
            ):
                wq_sb = w1.tile([128, DKT, 512], F32R, tag="wq")
                wk_sb = w1.tile([128, DKT, 512], F32R, tag="wk")
                for ch in range(CH):
                    b, s0 = ch // QC, (ch % QC) * 512
                    cs_sb = rope.tile([128, 512], F32, name="cs_c", tag="cs_c")
                    ss_sb = rope.tile([128, 512], F32, name="ss_c", tag="ss_c")
                    nc.scalar.dma_start(out=cs_sb, in_=cs_d.ap()[:, s0:s0 + 512])
                    nc.scalar.dma_start(out=ss_sb, in_=ss_d.ap()[:, s0:s0 + 512])
                    qps = [ps1.tile([128, 512], F32, name=f"qps{h}", tag=f"q{h}")
                           for h in range(HLOC)]
                    kps = [ps1.tile([128, 512], F32, name=f"kps{h}", tag=f"k{h}")
                           for h in range(HLOC)]
                    for dk in range(DKT):
                        if ch == 0:
                            we = nc.scalar if dk % 2 == 0 else nc.sync
                            wf = nc.sync if dk % 2 == 0 else nc.scalar
                            we.dma_start(out=wq_sb[:, dk, :], in_=wq_d.ap()[:, dk, :])
                            wf.dma_start(out=wk_sb[:, dk, :], in_=wk_d.ap()[:, dk, :])
                        xt = xt1.tile([128, 512], F32R, name="xt", tag="xt")
                        nc.sync.dma_start(
                            out=xt, in_=xt_d.ap()[dk, :, ch * 512:(ch + 1) * 512]
                        )
                        for h in range(HLOC):
                            nc.tensor.matmul(
                                qps[h], wq_sb[:, dk, h * 128:(h + 1) * 128], xt,
                                start=(dk == 0), stop=(dk == DKT - 1),
                            )
                        for h in range(HLOC):
                            nc.tensor.matmul(
                                kps[h], wk_sb[:, dk, h * 128:(h + 1) * 128], xt,
                                start=(dk == 0), stop=(dk == DKT - 1),
                            )
                    work = []
                    for h in range(HLOC):
                        for ps, dst in ((qps[h], qdr), (kps[h], kdr)):
                            pc = rope.tile([128, 512], F32, name="pc", tag="pc",
                                           bufs=4)
                            t1 = rope.tile([128, 512], F32, name="t1", tag="t1",
                                           bufs=8)
                            s1 = rope.tile([128, 512], F32, name="s1", tag="s1",
                                           bufs=2)
                            s1w = rope.tile([128, 512], F32, name="s1w", tag="s1w",
                                            bufs=8)
                            nc.vector.tensor_copy(pc, ps)
                            nc.vector.tensor_mul(t1, pc, cs_sb)
                            nc.vector.tensor_mul(s1, pc, ss_sb)
                            nc.scalar.dma_start(out=s1w[0:64, :], in_=s1[64:128, :])
                            nc.scalar.dma_start(out=s1w[64:128, :], in_=s1[0:64, :])
                            work.append((h, dst, t1, s1w))
                    for h, dst, t1, s1w in work:
                        rr = rope.tile([128, 512], F32R, name="rr", tag="rr", bufs=2)
                        nc.vector.tensor_add(rr, t1, s1w)
                        nc.scalar.dma_start(out=dst[(h, b)].ap()[:, s0:s0 + 512],
                                            in_=rr)

            # Phases 2+3
            hbs = [(b, h) for b in range(B) for h in range(HLOC)]
            with (
                tc.tile_pool(name="qkv", bufs=2) as qkv,
                tc.tile_pool(name="hold", bufs=1) as hold,
                tc.tile_pool(name="smp", bufs=2) as smp,
                tc.tile_pool(name="ps3", bufs=1, space="PSUM") as ps3,
                tc.tile_pool(name="ps4", bufs=3, space="PSUM") as ps4,
            ):
                def load_hb(i):
                    b, h = hbs[i]
                    qT = qkv.tile([128, S], F32R, name=f"qT_{i}", tag="qT")
                    kT = qkv.tile([128, S], F32R, name=f"kT_{i}", tag="kT")
                    vT = qkv.tile([128, KT, 128], F32R, name=f"vT_{i}", tag="vT")
                    vsrc = vdr[b].ap()[:, h * 128:(h + 1) * 128].rearrange(
                        "(n p) d -> p n d", p=128
                    )
                    for j in range(QC):
                        sl = slice(j * 512, (j + 1) * 512)
                        nc.sync.dma_start(out=qT[:, sl], in_=qdr[(h, b)].ap()[:, sl])
                        nc.sync.dma_start(out=kT[:, sl], in_=kdr[(h, b)].ap()[:, sl])
                        nc.sync.dma_start(
                            out=vT[:, j * 4:(j + 1) * 4, :],
                            in_=vsrc[:, j * 4:(j + 1) * 4, :],
                        )
                    return qT, kT, vT

                tiles = {0: load_hb(0)}
                wo_sb = hold.tile([128, HLOC, D], F32R, tag="wo")
                for h in range(HLOC):
                    nc.scalar.dma_start(out=wo_sb[:, h, :], in_=wo_d.ap()[:, h, :])

                hoTs = {}
                for i, (b, h) in enumerate(hbs):
                    if h == 0:
                        hoTs[b] = hold.tile([128, HLOC, S], F32R,
                                            name=f"hoT_{b}", tag=f"hoT{b}")
                    hoT = hoTs[b]
                    if i + 1 < len(hbs):
                        tiles[i + 1] = load_hb(i + 1)
                    qT, kT, vT = tiles.pop(i)
                    for qc in range(QC):
                        qs = qc * 512
                        sums = ps3.tile([1, 512], F32, name="sums", tag="sums")
                        hops = ps3.tile([128, 512], F32, name="hops", tag="hops")
                        for kt in range(KT):
                            st = ps4.tile([128, 512], F32, name="st", tag="st")
                            nc.tensor.matmul(
                                st, kT[:, kt * 128:(kt + 1) * 128],
                                qT[:, qs:qs + 512], start=True, stop=True,
                            )
                            mkt = smp.tile([128, 512], F32, name="mkt", tag="mkt")
                            nc.sync.dma_start(out=mkt, in_=mk_d.ap()[kt, :, qs:qs + 512])
                            nc.vector.tensor_add(st, st, mkt)
                            ex = smp.tile([128, 512], F32R, name="ex", tag="ex",
                                          bufs=4)
                            nc.scalar.activation(ex, st, EXP, scale=ISQRT)
                            nc.tensor.matmul(sums, ones_sb, ex, start=(kt == 0),
                                             stop=(kt == KT - 1))
                            nc.tensor.matmul(hops, vT[:, kt, :], ex, start=(kt == 0),
                                             stop=(kt == KT - 1))
                        recip = smp.tile([1, 512], F32, name="recip", tag="recip")
                        nc.vector.reciprocal(recip, sums)
                        bc = smp.tile([128, 512], F32, name="bc", tag="bc")
                        nc.gpsimd.partition_broadcast(bc, recip)
                        nc.vector.tensor_mul(hoT[:, h, qs:qs + 512], hops, bc)

                for b in range(B):
                    with (
                        tc.tile_pool(name=f"oc{b}", bufs=3) as ocp,
                        tc.tile_pool(name=f"ps5{b}", bufs=3, space="PSUM") as ps5,
                    ):
                        for t in range(S // 128):
                            for oc in range(D // 512):
                                ops = ps5.tile([128, 512], F32, name="ops", tag="ops")
                                for h in range(HLOC):
                                    nc.tensor.matmul(
                                        ops, hoTs[b][:, h, t * 128:(t + 1) * 128],
                                        wo_sb[:, h, oc * 512:(oc + 1) * 512],
                                        start=(h == 0), stop=(h == HLOC - 1),
                                    )
                                ot = ocp.tile([128, 512], F32, name="ot", tag="ot")
                                nc.vector.tensor_copy(ot, ops)
                                nc.scalar.dma_start(
                                    out=out_d.ap()[
                                        b * S + t * 128:b * S + (t + 1) * 128,
                                        oc * 512:(oc + 1) * 512,
                                    ],
                                    in_=ot,
                                )

    nc.compile()
    return nc


def _get_nc(causal: bool):
    if causal not in _CACHE:
        _CACHE[causal] = _build_causal() if causal else _build_legacy()
    return _CACHE[causal]


def _host_prep(x, wq, wk, wv, wo, freqs_cos, freqs_sin, mask):
    """Build per-core input maps."""
    x2 = np.ascontiguousarray(x.reshape(TOK, D).T)          # [D, TOK]
    xt = x2.reshape(DKT, 128, TOK)

    cs = np.concatenate([freqs_cos.T, freqs_cos.T], axis=0).astype(np.float32)
    ss = np.concatenate([freqs_sin.T, -freqs_sin.T], axis=0).astype(np.float32)

    m2 = np.asarray(mask, dtype=np.float32).reshape(S, S)
    tril = np.tril(np.ones((S, S), dtype=bool))
    causal = bool(np.all(m2[tril] == 0.0) and np.all(m2[~tril] <= -1e8))
    if causal:
        mk = np.ascontiguousarray(m2[:128, :128].T)         # [k,q] triangle
    else:
        mk = np.ascontiguousarray(m2.T.reshape(KT, 128, S))

    # per-head column permutation: evens then odds (RoPE rotate-half form)
    perm = np.concatenate([np.arange(0, HD, 2), np.arange(1, HD, 2)])

    in_maps = []
    for c in range(NC):
        cols = np.concatenate(
            [(4 * c + h) * HD + perm for h in range(HLOC)]
        )
        wq_c = np.ascontiguousarray(
            wq[:, cols].reshape(DKT, 128, 512).transpose(1, 0, 2)
        )
        wk_c = np.ascontiguousarray(
            wk[:, cols].reshape(DKT, 128, 512).transpose(1, 0, 2)
        )
        vcols = np.arange(4 * c * HD, 4 * (c + 1) * HD)
        wv_c = np.ascontiguousarray(
            wv[:, vcols].reshape(DKT, 128, 512).transpose(1, 0, 2)
        )
        wo_c = np.ascontiguousarray(
            wo[vcols, :].reshape(HLOC, 128, D).transpose(1, 0, 2)
        )
        m = {
            "xt": xt, "wq": wq_c, "wk": wk_c, "wv": wv_c, "wo": wo_c,
            "cs": cs, "ss": ss,
        }
        m["mtri" if causal else "maskf"] = mk
        in_maps.append(m)
    return in_maps, causal


def kernel(x, wq, wk, wv, wo, freqs_cos, freqs_sin, mask, **_unused):
    from concourse.bass_utils import run_bass_kernel_spmd

    x = np.asarray(x, dtype=np.float32)
    wq = np.asarray(wq, dtype=np.float32)
    wk = np.asarray(wk, dtype=np.float32)
    wv = np.asarray(wv, dtype=np.float32)
    wo = np.asarray(wo, dtype=np.float32)
    freqs_cos = np.asarray(freqs_cos, dtype=np.float32)
    freqs_sin = np.asarray(freqs_sin, dtype=np.float32)

    in_maps, causal = _host_prep(x, wq, wk, wv, wo, freqs_cos, freqs_sin, mask)
    nc = _get_nc(causal)
    res = run_bass_kernel_spmd(nc, in_maps, list(range(NC)))
    out = res.results[0]["out"].astype(np.float32)
    for c in range(1, NC):
        out = out + res.results[c]["out"].astype(np.float32)
    return out.reshape(B, S, D).astype(np.float32)
